# revision 1
# baseline (speedup 1.0000x reference)
"""Trainium2 Bass kernel: causal spatial attention block (nn_AttentionBlock).

Strategy: data-parallel over batch across 8 NeuronCores (4 batches per core,
no collectives). Per batch: QKV 1x1-conv projections as f32r matmuls,
causal attention computed in [t, s] (keys-on-partitions) orientation so the
probability tiles feed the A@V matmul with no transposes; V is produced
pre-transposed directly by the projection matmul orientation. Softmax
denominators come from a ones-matrix matmul (sum over t broadcast to all
partitions); 1/x is exp(-ln(x)) on the scalar engine (ACT Reciprocal is
banned for accuracy). f32r (TF32-like) matmuls run at 1 cycle/row for
free-dim >= 256; 128-wide diagonal tiles are widened to 256 and masked with
a [zeros | triu] block to stay on the fast path.
"""
import numpy as np
from contextlib import ExitStack

import concourse.bass as bass
import concourse.mybir as mybir
import concourse.tile as tile
from concourse import bacc
from concourse.bass_utils import run_bass_kernel_spmd

F32 = mybir.dt.float32
F32R = mybir.dt.float32r
AF = mybir.ActivationFunctionType
ALU = mybir.AluOpType

B, C, L, EMB = 32, 224, 32, 16
S = L * L            # 1024
CIN = 256
NCORES = 8
NB = B // NCORES     # 4 batches per core


def _pos_embeddings() -> np.ndarray:
    """[2E, S] positional-embedding channels, replicating the reference
    (raw row-major reshape of the [L, E] table, NOT a transpose)."""
    pos = np.arange(L)[:, None].astype(np.float64)
    j = np.arange(EMB)[None, :]
    enc = pos / np.power(10000.0, 2.0 * (j // 2) / EMB)
    enc[0, :] = 0.0
    enc[1:, 0::2] = np.sin(enc[1:, 0::2])
    enc[1:, 1::2] = np.cos(enc[1:, 1::2])
    t = enc.astype(np.float32)                            # [L, E]
    x = np.tile(t.reshape(1, EMB, L, 1), (1, 1, 1, L))
    y = np.tile(t.reshape(1, EMB, 1, L), (1, 1, L, 1))
    pe = np.concatenate((x, y), axis=1)[0]                # [2E, L, L]
    return np.ascontiguousarray(pe.reshape(2 * EMB, S))


def _pin_act_tables():
    """Make Bacc's table picker see only the natural_log_exp_and_others set
    (it holds exp+ln+relu+copy+identity — everything this kernel uses) so a
    single ACT table load serves the whole kernel instead of thrashing
    between exp_and_others and natural_log (~1.3 us per reload)."""
    from concourse import bacc as _bacc
    real = _bacc.get_activation_tables
    def patched(arch):
        tables = real(arch)
        keep = "natural_log_exp_and_others"
        assert keep in tables
        return {name: (funcs if name == keep else set())
                for name, funcs in tables.items()}
    _bacc.get_activation_tables = patched
    return real


def build(reps: int = 1):
    """Build + finalize the per-core Bass program (same program on all 8).

    reps > 1 repeats the whole per-core computation back-to-back inside one
    NEFF — used by the timing harness to amortize launch overhead."""
    real_tables = _pin_act_tables()
    nc = bacc.Bacc("TRN2", target_bir_lowering=False, debug=False,
                   num_devices=NCORES)
    x_d = nc.declare_dram_parameter("x", [NB, C, S], F32, isOutput=False)
    pe_d = nc.declare_dram_parameter("pe", [2 * EMB, S], F32, isOutput=False)
    wq_d = nc.declare_dram_parameter("wqt", [CIN, 256], F32, isOutput=False)
    wk_d = nc.declare_dram_parameter("wkt", [CIN, 256], F32, isOutput=False)
    wv_d = nc.declare_dram_parameter("wvt", [CIN, 256], F32, isOutput=False)
    # bqk: [128, 4] = [bq half0 | bq half1 | bk half0 | bk half1] columns
    bqk_d = nc.declare_dram_parameter("bqk", [128, 4], F32, isOutput=False)
    bv_d = nc.declare_dram_parameter("bv", [256], F32, isOutput=False)
    # mask[:, 0:128] = zeros (unused now), mask[:, 128:256] = triu (t <= s)
    mk_d = nc.declare_dram_parameter("mask", [128, 256], F32, isOutput=False)
    out_d = nc.declare_dram_parameter("out", [NB, 256, S], F32, isOutput=True)

    with ExitStack() as ctx:
        tc = ctx.enter_context(tile.TileContext(nc))
        const = ctx.enter_context(tc.tile_pool(name="const", bufs=1))
        xp = ctx.enter_context(tc.tile_pool(name="x0", bufs=2))
        qkp = ctx.enter_context(tc.tile_pool(name="qk", bufs=2))
        vtp = ctx.enter_context(tc.tile_pool(name="vt", bufs=2))
        pp = ctx.enter_context(tc.tile_pool(name="p", bufs=8))
        ep = ctx.enter_context(tc.tile_pool(name="epi", bufs=4))
        # work: projection + score psum share 5 banks; acc: o0/o1/den 3 banks
        ps_w = ctx.enter_context(tc.tile_pool(name="work", bufs=5, space="PSUM"))
        ps_acc = ctx.enter_context(tc.tile_pool(name="acc", bufs=3, space="PSUM"))

        # -------- batch-0 x load first: it heads the serial DGE queue.
        # Each channel-half is loaded (and f32r-rounded) in two 512-col
        # s-halves so the first projection/attention consumers wait for
        # only a quarter of the batch's data.
        def load_x0(b):
            if "wv" not in consts:
                # wv first: the vT projection is the first compute
                _load_wv()
            x0f = xp.tile([128, 2, S], F32, tag="x0f")
            x0r = xp.tile([128, 2, S], F32R, tag="x0r")
            for h0, h1 in ((0, 512), (512, S)):
                nc.sync.dma_start(x0f[:, 0, h0:h1], x_d[b, 0:128, h0:h1])
                nc.sync.dma_start(x0f[0:96, 1, h0:h1], x_d[b, 128:224, h0:h1])
                nc.sync.dma_start(x0f[96:128, 1, h0:h1], pe_d[:, h0:h1])
                nc.vector.tensor_copy(x0r[:, 0, h0:h1], x0f[:, 0, h0:h1])
                nc.vector.tensor_copy(x0r[:, 1, h0:h1], x0f[:, 1, h0:h1])
            if "wq" not in consts:
                _load_weights()
            return x0r

        consts = {}

        def round_from_dram(dram_ap, shape, name, eng="scalar"):
            f = const.tile(shape, F32, tag=name + "_f")
            nc.sync.dma_start(f[:], dram_ap)
            r = const.tile(shape, F32R, tag=name + "_r")
            if eng == "scalar":
                nc.scalar.copy(r[:], f[:])
            else:
                nc.vector.tensor_copy(r[:], f[:])
            return r

        def _load_wv():
            consts["wv"] = round_from_dram(
                wv_d[:].rearrange("(ci p) m -> p ci m", p=128),
                [128, 2, 256], "wv", eng="vector")
            consts["bv"] = round_from_dram(bv_d[:].unsqueeze(0), [1, 256],
                                           "bv", eng="vector")

        def _make_ones_and_warm():
            # ones needs no DMA: build it immediately, then run throwaway
            # matmuls on it while the startup DMAs stream, so the PE's HAM
            # clock gate is already at 2.4 GHz when real work arrives
            ones_f = const.tile([128, 128], F32, tag="ones_f")
            nc.vector.memset(ones_f[:], 1.0)
            ones_r = const.tile([128, 128], F32R, tag="ones_r")
            nc.vector.tensor_copy(ones_r[:], ones_f[:])
            consts["ones"] = ones_r
            warm = ps_acc.tile([128, 128], F32, tag="acc")
            for _ in range(8):
                nc.tensor.matmul(warm[:], ones_r[:], ones_r[:],
                                 start=True, stop=True)

        def _load_weights():
            consts["wq"] = round_from_dram(
                wq_d[:].rearrange("(ci p) m -> p ci m", p=128),
                [128, 2, 256], "wq")
            consts["wk"] = round_from_dram(
                wk_d[:].rearrange("(ci p) m -> p ci m", p=128),
                [128, 2, 256], "wk")
            consts["mask"] = round_from_dram(mk_d[:], [128, 256], "mask",
                                             eng="vector")
            bqk = const.tile([128, 4], F32, tag="bqk")
            nc.sync.dma_start(bqk[:], bqk_d[:])
            consts["bqk"] = bqk

        _make_ones_and_warm()
        x0r_next = load_x0(0)

        # ---------------- per-batch pipeline ----------------
        seq = [b for _ in range(reps) for b in range(NB)]
        for bi, b in enumerate(seq):
            is_last = (bi == len(seq) - 1)
            x0r = x0r_next
            if not is_last:
                x0r_next = load_x0(seq[bi + 1])

            wq_r, wk_r, wv_r = consts["wq"], consts["wk"], consts["wv"]
            mask_r, bv_r, bqk = consts["mask"], consts["bv"], consts["bqk"]
            ones_r = consts["ones"]

            # V transposed: vT[t, v] = relu(x0.T @ wvT + bv); relus
            # alternate between DVE and ACT so neither engine becomes the
            # drain bottleneck for the work-psum slots.
            vt_r = vtp.tile([128, 8, 256], F32R, tag="vt")
            vt_done = [0]

            def ensure_vt(n):
                for ti in range(vt_done[0], n):
                    ps = ps_w.tile([128, 256], F32, tag="work")
                    for ci in range(2):
                        nc.tensor.matmul(
                            ps[:],
                            x0r[:, ci, ti * 128:(ti + 1) * 128],
                            wv_r[:, ci, :],
                            start=(ci == 0), stop=False)
                    # + bv along the free dim via rank-1 ones x bv update
                    nc.tensor.matmul(ps[:], ones_r[0:1, :], bv_r[0:1, :],
                                     start=False, stop=True)
                    if ti % 2 == 0:
                        nc.vector.tensor_scalar(vt_r[:, ti, :], ps[:], 0.0,
                                                None, op0=ALU.max)
                    else:
                        nc.scalar.activation(vt_r[:, ti, :], ps[:], AF.Relu)
                vt_done[0] = max(vt_done[0], n)

            ensure_vt(8)

            # Q, K projections: q[c_out, s] = relu(wT.T @ x0 + b)
            q_r = qkp.tile([128, 2, S], F32R, tag="q")
            k_r = qkp.tile([128, 2, S], F32R, tag="k")
            for sj in range(2):
                for pi, (wr, dst) in enumerate(((wq_r, q_r), (wk_r, k_r))):
                    for m in range(2):
                        ps = ps_w.tile([128, 512], F32, tag="work")
                        for ci in range(2):
                            nc.tensor.matmul(
                                ps[:],
                                wr[:, ci, m * 128:(m + 1) * 128],
                                x0r[:, ci, sj * 512:(sj + 1) * 512],
                                start=(ci == 0), stop=(ci == 1))
                        # relu(x + b): bias is per-partition; alternate the
                        # consuming engine so the work-psum slots drain via
                        # both DVE and ACT during the projection burst
                        bias_ap = bqk[:, 2 * pi + m:2 * pi + m + 1]
                        dst_ap = dst[:, m, sj * 512:(sj + 1) * 512]
                        if m == 1:
                            nc.scalar.activation(dst_ap, ps[:], AF.Relu,
                                                 bias=bias_ap)
                        else:
                            nc.vector.tensor_scalar(
                                dst_ap, ps[:], bias_ap, 0.0,
                                op0=ALU.add, op1=ALU.max)

            # causal attention over s-chunks; the final batch splits its last
            # 512 columns into two 256-wide chunks so the kernel tail
            # (epilogue of the very last chunk) is half as long
            chunks = [(0, 512), (512, 768), (768, 1024)] if is_last \
                     else [(0, 512), (512, 1024)]
            for (ck0, ck1) in chunks:
                cw = ck1 - ck0
                nt = ck1 // 128
                ensure_vt(nt)
                o0 = ps_acc.tile([128, cw], F32, tag="acc")
                o1 = ps_acc.tile([128, cw], F32, tag="acc")
                dn = ps_acc.tile([128, cw], F32, tag="acc")
                for ti in range(nt):
                    diag = (ti * 128 >= ck0)
                    w = ck1 - max(ck0, ti * 128)
                    we = max(w, 256)       # f32r needs N>=256 for 1 cyc/row
                    cs = ck1 - we
                    loc = cw - we
                    sp = ps_w.tile([128, 512], F32, tag="work")
                    for ci in range(2):
                        nc.tensor.matmul(
                            sp[:, :we],
                            k_r[:, ci, ti * 128:(ti + 1) * 128],
                            q_r[:, ci, cs:cs + we],
                            start=(ci == 0), stop=(ci == 1))
                    p = pp.tile([128, 512], F32R, tag="p")
                    nc.scalar.activation(p[:, :we], sp[:, :we], AF.Exp,
                                         scale=0.0625)
                    if diag and we > w:
                        # widened tile: [zeros | triu] masks both the dead
                        # left half and the diagonal block in one op
                        nc.vector.tensor_tensor(p[:, 0:256], p[:, 0:256],
                                                mask_r[:], op=ALU.mult)
                    elif diag:
                        nc.vector.tensor_tensor(p[:, 0:128], p[:, 0:128],
                                                mask_r[:, 128:256],
                                                op=ALU.mult)
                    first, last = (ti == 0), (ti == nt - 1)
                    # denominator first: its consumer (Ln) gates the epilogue
                    nc.tensor.matmul(dn[:, loc:loc + we], ones_r[:, :],
                                     p[:, :we], start=first, stop=last)
                    nc.tensor.matmul(o0[:, loc:loc + we], vt_r[:, ti, 0:128],
                                     p[:, :we], start=first, stop=last)
                    nc.tensor.matmul(o1[:, loc:loc + we], vt_r[:, ti, 128:256],
                                     p[:, :we], start=first, stop=last)
                # 1/den = exp(-ln(den)); ACT Reciprocal is banned (accuracy)
                # last batch: no more input prefetch — use the fast sync DGE
                dma_eng = nc.sync if is_last else nc.gpsimd
                lnt = ep.tile([128, 512], F32, tag="lnt")
                rec = ep.tile([128, 512], F32, tag="rec")
                nc.scalar.activation(lnt[:, :cw], dn[:], AF.Ln)
                nc.scalar.activation(rec[:, :cw], lnt[:, :cw], AF.Exp,
                                     scale=-1.0)
                for m, om in enumerate((o0, o1)):
                    osb = ep.tile([128, 512], F32, tag="osb")
                    nc.vector.tensor_tensor(osb[:, :cw], om[:], rec[:, :cw],
                                            op=ALU.mult)
                    # outputs ride the Pool engine's DGE mid-kernel to
                    # keep the sync queue free for input prefetch
                    dma_eng.dma_start(
                        out_d[b, m * 128:(m + 1) * 128, ck0:ck1],
                        osb[:, :cw])

    try:
        nc.finalize()
    finally:
        from concourse import bacc as _bacc
        _bacc.get_activation_tables = real_tables
    return nc


def make_in_maps(x, wq, bq, wk, bk, wv, bv):
    x_r = np.ascontiguousarray(x.reshape(B, C, S).astype(np.float32))
    pe = _pos_embeddings()
    wqt = np.ascontiguousarray(wq.T.astype(np.float32))
    wkt = np.ascontiguousarray(wk.T.astype(np.float32))
    wvt = np.ascontiguousarray(wv.T.astype(np.float32))
    bq = bq.astype(np.float32)
    bk = bk.astype(np.float32)
    bqk = np.ascontiguousarray(
        np.stack([bq[:128], bq[128:], bk[:128], bk[128:]], axis=1))
    mask = np.concatenate([np.zeros((128, 128), np.float32),
                           np.triu(np.ones((128, 128), np.float32))], axis=1)
    common = dict(pe=pe, wqt=wqt, wkt=wkt, wvt=wvt, bqk=bqk,
                  bv=np.ascontiguousarray(bv.astype(np.float32)),
                  mask=mask)
    return [dict(x=np.ascontiguousarray(x_r[i * NB:(i + 1) * NB]), **common)
            for i in range(NCORES)]


_NC_CACHE = None


def kernel(x, wq, bq, wk, bk, wv, bv):
    global _NC_CACHE
    if _NC_CACHE is None:
        _NC_CACHE = build()
    nc = _NC_CACHE
    in_maps = make_in_maps(x, wq, bq, wk, bk, wv, bv)
    res = run_bass_kernel_spmd(nc, in_maps, core_ids=list(range(NCORES)))
    out = np.concatenate([res.results[i]["out"] for i in range(NCORES)], axis=0)
    return np.ascontiguousarray(out.reshape(B, 256, L, L).astype(np.float32))

